# revision 11
# baseline (speedup 1.0000x reference)
"""Trainium2 Bass kernel for nn_DRAM_MAC_temporal_encoding (polynomial attention).

Math (QK_mul=1):
    out = sum_i coef_i * (x @ (y-OFF)^i) * decay
        = (x * decay) @ P(y-OFF)            # P = Horner cubic, elementwise
so the whole problem is ONE [S,64]@[64,S] matmul per (b,h) head plus the
output write -> memory-bound. The tiny elementwise prep (poly on y,
row-scaling x, transposes, fp16 casts) runs on host; the device does
matmuls + store.

Precision: tolerance is rel_err < 2e-2. fp16 inputs + single fp16 matmul
(fp32 PSUM accumulate) + fp16 output measures 2.5e-4 on the numpy model —
so no hi/lo split and, crucially, the 50 MiB/core fp32 output write
becomes 25 MiB fp16 (host upcasts back to fp32). PSUM->SBUF fp32->fp16
drains rotate across Vector/Scalar/Pool so no single engine bottlenecks.

QK_mul=0: out = sum_i coef_i * ((x*d^i) @ (y-OFF)^i) -> two K=128 chunks
(4 stacked K=64 terms), same kernel with n_chunks=2.

Sharding: 24 (b,h) heads -> 3 per core across 8 cores.
"""

import ml_dtypes
import numpy as np

import concourse.mybir as mybir
import concourse.tile as tile
from concourse import bacc
from concourse.bass_utils import run_bass_kernel_spmd

C = [0.17393044, 0.15653739, 0.14088365, 0.12679529, 5.51975209,
     4.96777688, 4.4709992, -1.44776001, -1.30298401, 46.05483778]
MAX_ORDER = 3
X_MAX = 0.9
OFFSET = 0.45

B, H, S, D = 2, 12, 2048, 64
BH = B * H
N_CORES = 8
BLK = BH // N_CORES  # heads per core

M_TILE = 128   # output rows per matmul (PSUM partitions)
N_TILE = 512   # output cols per matmul (one fp32 PSUM bank)

_NC_CACHE = {}
_last_nc = None
_last_in_maps = None


def _coefs():
    cs = []
    idx = 0
    for i in range(MAX_ORDER + 1):
        n_j = MAX_ORDER - i + 1
        cs.append(sum(C[idx + j] * X_MAX ** j for j in range(n_j)))
        idx += n_j
    return cs  # [c0, c1, c2, c3]


def _build_nc(n_chunks, wk):
    """Device kernel: per core, BLK independent [S,S] fp16 output blocks,
    each output tile = sum over n_chunks K=128 bf16 matmuls.

    K=64 matmuls stream at ~1/3 the K=128 rate on TRN2 HW (630ns vs 233ns
    per [128,512]), so the contraction is always presented as K=128:
    a carries [hi; lo] bf16 rows, and when wk == 64 the w rows are
    replicated in SBUF (two DMAs from the same DRAM region) so one matmul
    computes (a_hi + a_lo) @ w."""
    nc = bacc.Bacc(None, target_bir_lowering=False)
    a_d = nc.dram_tensor("a", [BLK, n_chunks, 128, S], mybir.dt.bfloat16,
                         kind="ExternalInput")
    w_d = nc.dram_tensor("w", [BLK, n_chunks, wk, S], mybir.dt.bfloat16,
                         kind="ExternalInput")
    out_d = nc.dram_tensor("out", [BLK, S, S], mybir.dt.float16,
                           kind="ExternalOutput")

    with tile.TileContext(nc) as tc:
        with (
            tc.tile_pool(name="inp", bufs=1) as inp,
            tc.tile_pool(name="ps", bufs=2, space="PSUM") as psp,
            tc.tile_pool(name="outp", bufs=10) as outp,
        ):
            # Prefetch every input tile up front so the steady-state DMA
            # queues carry only output stores.
            a_ts, w_ts = {}, {}
            for blk in range(BLK):
                for c in range(n_chunks):
                    ta = inp.tile([128, S], mybir.dt.bfloat16,
                                  tag=f"a{blk}_{c}")
                    nc.sync.dma_start(ta[:], a_d[blk, c])
                    a_ts[(blk, c)] = ta
                    tw = inp.tile([128, S], mybir.dt.bfloat16,
                                  tag=f"w{blk}_{c}")
                    if wk == 64:
                        nc.sync.dma_start(tw[:64], w_d[blk, c])
                        nc.sync.dma_start(tw[64:], w_d[blk, c])
                    else:
                        nc.sync.dma_start(tw[:], w_d[blk, c])
                    w_ts[(blk, c)] = tw

            # Pool/GpSimd can't read PSUM on TRN2, so each row-tile drain is
            # split column-wise between DVE (0.96 GHz) and Act (1.2 GHz),
            # sized so both halves finish together (~1.1us) — halving the
            # per-tile drain latency keeps the store queue fed.
            V_COLS = 896
            with nc.allow_low_precision(reason="fp16 out within 2e-2 tol"):
                for blk in range(BLK):
                    for st in range(S // M_TILE):
                        ps = psp.tile([M_TILE, S], mybir.dt.float32, tag="ps")
                        for nt in range(S // N_TILE):
                            for c in range(n_chunks):
                                nc.tensor.matmul(
                                    ps[:, nt * N_TILE:(nt + 1) * N_TILE],
                                    a_ts[(blk, c)][
                                        :, st * M_TILE:(st + 1) * M_TILE],
                                    w_ts[(blk, c)][
                                        :, nt * N_TILE:(nt + 1) * N_TILE],
                                    start=(c == 0),
                                    stop=(c == n_chunks - 1),
                                )
                        ot = outp.tile([M_TILE, S], mybir.dt.float16,
                                       tag="ot")
                        nc.vector.tensor_copy(ot[:, :V_COLS], ps[:, :V_COLS])
                        nc.scalar.copy(ot[:, V_COLS:], ps[:, V_COLS:])
                        nc.sync.dma_start(
                            out_d[blk, st * M_TILE:(st + 1) * M_TILE, :],
                            ot[:])
    nc.compile()
    return nc


def _get_nc(n_chunks, wk):
    key = (n_chunks, wk)
    if key not in _NC_CACHE:
        _NC_CACHE[key] = _build_nc(n_chunks, wk)
    return _NC_CACHE[key]


def _hilo(v):
    """f32 -> stacked [hi; lo] bf16 rows along axis -2 (hi+lo ~= v)."""
    hi = v.astype(ml_dtypes.bfloat16)
    lo = (v - hi.astype(np.float32)).astype(ml_dtypes.bfloat16)
    return np.concatenate([hi, lo], axis=-2)


def _prepare(x, y, dm, qk):
    """Host prep -> (a, w) bf16 arrays: a [BH, n_chunks, 128, S],
    w [BH, n_chunks, wk, S]."""
    c0, c1, c2, c3 = _coefs()
    yo = (y - OFFSET).astype(np.float32)  # [B,H,D,S]
    if qk:
        n_chunks, wk = 1, D
        at = np.ascontiguousarray(
            (x * dm[None, None, :, :]).transpose(0, 1, 3, 2)
        ).reshape(BH, D, S)
        a = _hilo(at).reshape(BH, 1, 2 * D, S)
        w = (((c3 * yo + c2) * yo + c1) * yo + c0) \
            .astype(ml_dtypes.bfloat16).reshape(BH, 1, D, S)
    else:
        n_chunks, wk = 2, 2 * D
        d = dm[:, 0]
        a = np.empty((BH, 2, 2 * D, S), dtype=ml_dtypes.bfloat16)
        w = np.empty((BH, 2, 2 * D, S), dtype=ml_dtypes.bfloat16)
        xt = x.transpose(0, 1, 3, 2).reshape(BH, D, S)
        di = np.ones_like(d)
        yi = np.ones_like(yo).reshape(BH, D, S)
        yo_r = yo.reshape(BH, D, S)
        for i, ci in enumerate((c0, c1, c2, c3)):
            c, half = divmod(i, 2)
            a[:, c, half * D:(half + 1) * D] = xt * di[None, None, :]
            w[:, c, half * D:(half + 1) * D] = ci * yi
            di = di * d
            yi = yi * yo_r
    return a, w, n_chunks, wk


def kernel(**inputs):
    x = np.asarray(inputs["x"], dtype=np.float32)
    y = np.asarray(inputs["y"], dtype=np.float32)
    dm = np.asarray(inputs["decay_mask"], dtype=np.float32)
    qk = int(np.asarray(inputs["QK_mul"]))

    a, w, n_chunks, wk = _prepare(x, y, dm, qk)
    nc = _get_nc(n_chunks, wk)

    in_maps = [
        {"a": a[c * BLK:(c + 1) * BLK], "w": w[c * BLK:(c + 1) * BLK]}
        for c in range(N_CORES)
    ]
    global _last_nc, _last_in_maps
    _last_nc, _last_in_maps = nc, in_maps

    res = None
    for attempt in range(3):
        try:
            res = run_bass_kernel_spmd(nc, in_maps,
                                       core_ids=list(range(N_CORES)))
            break
        except Exception:
            # transient NRT_EXEC_UNIT_UNRECOVERABLE wedges occur on busy axon
            # terminals; they clear after a pause
            if attempt == 2:
                raise
            import time
            time.sleep(45)

    out = np.empty((BH, S, S), dtype=np.float32)
    for c in range(N_CORES):
        out[c * BLK:(c + 1) * BLK] = res.results[c]["out"]
    return out.reshape(B, H, S, S)


# revision 12
# speedup vs baseline: 1.0068x; 1.0068x over previous
"""Trainium2 Bass kernel for nn_DRAM_MAC_temporal_encoding (polynomial attention).

Math (QK_mul=1):
    out = sum_i coef_i * (x @ (y-OFF)^i) * decay
        = (x * decay) @ P(y-OFF)            # P = Horner cubic, elementwise
so the whole problem is ONE [S,64]@[64,S] matmul per (b,h) head plus the
output write -> memory-bound. The tiny elementwise prep (poly on y,
row-scaling x, transposes, fp16 casts) runs on host; the device does
matmuls + store.

Precision: tolerance is rel_err < 2e-2. fp16 inputs + single fp16 matmul
(fp32 PSUM accumulate) + fp16 output measures 2.5e-4 on the numpy model —
so no hi/lo split and, crucially, the 50 MiB/core fp32 output write
becomes 25 MiB fp16 (host upcasts back to fp32). PSUM->SBUF fp32->fp16
drains rotate across Vector/Scalar/Pool so no single engine bottlenecks.

QK_mul=0: out = sum_i coef_i * ((x*d^i) @ (y-OFF)^i) -> two K=128 chunks
(4 stacked K=64 terms), same kernel with n_chunks=2.

Sharding: 24 (b,h) heads -> 3 per core across 8 cores.
"""

import ml_dtypes
import numpy as np

import concourse.mybir as mybir
import concourse.tile as tile
from concourse import bacc
from concourse.bass_utils import run_bass_kernel_spmd

C = [0.17393044, 0.15653739, 0.14088365, 0.12679529, 5.51975209,
     4.96777688, 4.4709992, -1.44776001, -1.30298401, 46.05483778]
MAX_ORDER = 3
X_MAX = 0.9
OFFSET = 0.45

B, H, S, D = 2, 12, 2048, 64
BH = B * H
N_CORES = 8
BLK = BH // N_CORES  # heads per core

M_TILE = 128   # output rows per matmul (PSUM partitions)
N_TILE = 512   # output cols per matmul (one fp32 PSUM bank)

_NC_CACHE = {}
_last_nc = None
_last_in_maps = None


def _coefs():
    cs = []
    idx = 0
    for i in range(MAX_ORDER + 1):
        n_j = MAX_ORDER - i + 1
        cs.append(sum(C[idx + j] * X_MAX ** j for j in range(n_j)))
        idx += n_j
    return cs  # [c0, c1, c2, c3]


def _build_nc(n_chunks, wk):
    """Device kernel: per core, BLK independent [S,S] fp16 output blocks,
    each output tile = sum over n_chunks K=128 bf16 matmuls.

    K=64 matmuls stream at ~1/3 the K=128 rate on TRN2 HW (630ns vs 233ns
    per [128,512]), so the contraction is always presented as K=128:
    a carries [hi; lo] bf16 rows, and when wk == 64 the w rows are
    replicated in SBUF (two DMAs from the same DRAM region) so one matmul
    computes (a_hi + a_lo) @ w."""
    nc = bacc.Bacc(None, target_bir_lowering=False)
    a_d = nc.dram_tensor("a", [BLK, n_chunks, 128, S], mybir.dt.bfloat16,
                         kind="ExternalInput")
    w_d = nc.dram_tensor("w", [BLK, n_chunks, wk, S], mybir.dt.bfloat16,
                         kind="ExternalInput")
    out_d = nc.dram_tensor("out", [BLK, S, S], mybir.dt.float16,
                           kind="ExternalOutput")

    with tile.TileContext(nc) as tc:
        with (
            tc.tile_pool(name="inp", bufs=1) as inp,
            tc.tile_pool(name="ps", bufs=2, space="PSUM") as psp,
            tc.tile_pool(name="outp", bufs=10) as outp,
        ):
            # Prefetch every input tile up front so the steady-state DMA
            # queues carry only output stores.
            a_ts, w_ts = {}, {}
            for blk in range(BLK):
                for c in range(n_chunks):
                    ta = inp.tile([128, S], mybir.dt.bfloat16,
                                  tag=f"a{blk}_{c}")
                    nc.sync.dma_start(ta[:], a_d[blk, c])
                    a_ts[(blk, c)] = ta
                    tw = inp.tile([128, S], mybir.dt.bfloat16,
                                  tag=f"w{blk}_{c}")
                    if wk == 64:
                        nc.sync.dma_start(tw[:64], w_d[blk, c])
                        nc.sync.dma_start(tw[64:], w_d[blk, c])
                    else:
                        nc.sync.dma_start(tw[:], w_d[blk, c])
                    w_ts[(blk, c)] = tw

            # Pool/GpSimd can't read PSUM on TRN2, so each row-tile drain is
            # split column-wise between DVE (0.96 GHz) and Act (1.2 GHz),
            # split at the PSUM bank boundary (cols are 512-wide fp32 banks;
            # unaligned splits contend on a shared bank) — halving the
            # per-tile drain latency keeps the store queue fed.
            V_COLS = 1024
            with nc.allow_low_precision(reason="fp16 out within 2e-2 tol"):
                for blk in range(BLK):
                    for st in range(S // M_TILE):
                        ps = psp.tile([M_TILE, S], mybir.dt.float32, tag="ps")
                        for nt in range(S // N_TILE):
                            for c in range(n_chunks):
                                nc.tensor.matmul(
                                    ps[:, nt * N_TILE:(nt + 1) * N_TILE],
                                    a_ts[(blk, c)][
                                        :, st * M_TILE:(st + 1) * M_TILE],
                                    w_ts[(blk, c)][
                                        :, nt * N_TILE:(nt + 1) * N_TILE],
                                    start=(c == 0),
                                    stop=(c == n_chunks - 1),
                                )
                        ot = outp.tile([M_TILE, S], mybir.dt.float16,
                                       tag="ot")
                        nc.vector.tensor_copy(ot[:, :V_COLS], ps[:, :V_COLS])
                        nc.scalar.copy(ot[:, V_COLS:], ps[:, V_COLS:])
                        nc.sync.dma_start(
                            out_d[blk, st * M_TILE:(st + 1) * M_TILE, :],
                            ot[:])
    nc.compile()
    return nc


def _get_nc(n_chunks, wk):
    key = (n_chunks, wk)
    if key not in _NC_CACHE:
        _NC_CACHE[key] = _build_nc(n_chunks, wk)
    return _NC_CACHE[key]


def _hilo(v):
    """f32 -> stacked [hi; lo] bf16 rows along axis -2 (hi+lo ~= v)."""
    hi = v.astype(ml_dtypes.bfloat16)
    lo = (v - hi.astype(np.float32)).astype(ml_dtypes.bfloat16)
    return np.concatenate([hi, lo], axis=-2)


def _prepare(x, y, dm, qk):
    """Host prep -> (a, w) bf16 arrays: a [BH, n_chunks, 128, S],
    w [BH, n_chunks, wk, S]."""
    c0, c1, c2, c3 = _coefs()
    yo = (y - OFFSET).astype(np.float32)  # [B,H,D,S]
    if qk:
        n_chunks, wk = 1, D
        at = np.ascontiguousarray(
            (x * dm[None, None, :, :]).transpose(0, 1, 3, 2)
        ).reshape(BH, D, S)
        a = _hilo(at).reshape(BH, 1, 2 * D, S)
        w = (((c3 * yo + c2) * yo + c1) * yo + c0) \
            .astype(ml_dtypes.bfloat16).reshape(BH, 1, D, S)
    else:
        n_chunks, wk = 2, 2 * D
        d = dm[:, 0]
        a = np.empty((BH, 2, 2 * D, S), dtype=ml_dtypes.bfloat16)
        w = np.empty((BH, 2, 2 * D, S), dtype=ml_dtypes.bfloat16)
        xt = x.transpose(0, 1, 3, 2).reshape(BH, D, S)
        di = np.ones_like(d)
        yi = np.ones_like(yo).reshape(BH, D, S)
        yo_r = yo.reshape(BH, D, S)
        for i, ci in enumerate((c0, c1, c2, c3)):
            c, half = divmod(i, 2)
            a[:, c, half * D:(half + 1) * D] = xt * di[None, None, :]
            w[:, c, half * D:(half + 1) * D] = ci * yi
            di = di * d
            yi = yi * yo_r
    return a, w, n_chunks, wk


def kernel(**inputs):
    x = np.asarray(inputs["x"], dtype=np.float32)
    y = np.asarray(inputs["y"], dtype=np.float32)
    dm = np.asarray(inputs["decay_mask"], dtype=np.float32)
    qk = int(np.asarray(inputs["QK_mul"]))

    a, w, n_chunks, wk = _prepare(x, y, dm, qk)
    nc = _get_nc(n_chunks, wk)

    in_maps = [
        {"a": a[c * BLK:(c + 1) * BLK], "w": w[c * BLK:(c + 1) * BLK]}
        for c in range(N_CORES)
    ]
    global _last_nc, _last_in_maps
    _last_nc, _last_in_maps = nc, in_maps

    res = None
    for attempt in range(3):
        try:
            res = run_bass_kernel_spmd(nc, in_maps,
                                       core_ids=list(range(N_CORES)))
            break
        except Exception:
            # transient NRT_EXEC_UNIT_UNRECOVERABLE wedges occur on busy axon
            # terminals; they clear after a pause
            if attempt == 2:
                raise
            import time
            time.sleep(45)

    out = np.empty((BH, S, S), dtype=np.float32)
    for c in range(N_CORES):
        out[c * BLK:(c + 1) * BLK] = res.results[c]["out"]
    return out.reshape(B, H, S, S)


# revision 13
# speedup vs baseline: 1.1091x; 1.1016x over previous
"""Trainium2 Bass kernel for nn_DRAM_MAC_temporal_encoding (polynomial attention).

Math (QK_mul=1):
    out = sum_i coef_i * (x @ (y-OFF)^i) * decay
        = (x * decay) @ P(y-OFF)            # P = Horner cubic, elementwise
so the whole problem is ONE [S,64]@[64,S] matmul per (b,h) head plus the
output write -> memory-bound. The tiny elementwise prep (poly on y,
row-scaling x, transposes, fp16 casts) runs on host; the device does
matmuls + store.

Precision: tolerance is rel_err < 2e-2. fp16 inputs + single fp16 matmul
(fp32 PSUM accumulate) + fp16 output measures 2.5e-4 on the numpy model —
so no hi/lo split and, crucially, the 50 MiB/core fp32 output write
becomes 25 MiB fp16 (host upcasts back to fp32). PSUM->SBUF fp32->fp16
drains rotate across Vector/Scalar/Pool so no single engine bottlenecks.

QK_mul=0: out = sum_i coef_i * ((x*d^i) @ (y-OFF)^i) -> two K=128 chunks
(4 stacked K=64 terms), same kernel with n_chunks=2.

Sharding: 24 (b,h) heads -> 3 per core across 8 cores.
"""

import ml_dtypes
import numpy as np

import concourse.mybir as mybir
import concourse.tile as tile
from concourse import bacc
from concourse.bass_utils import run_bass_kernel_spmd

C = [0.17393044, 0.15653739, 0.14088365, 0.12679529, 5.51975209,
     4.96777688, 4.4709992, -1.44776001, -1.30298401, 46.05483778]
MAX_ORDER = 3
X_MAX = 0.9
OFFSET = 0.45

B, H, S, D = 2, 12, 2048, 64
BH = B * H
N_CORES = 8
BLK = BH // N_CORES  # heads per core

M_TILE = 128   # output rows per matmul (PSUM partitions)
N_TILE = 512   # output cols per matmul (one fp32 PSUM bank)

_NC_CACHE = {}
_last_nc = None
_last_in_maps = None


def _coefs():
    cs = []
    idx = 0
    for i in range(MAX_ORDER + 1):
        n_j = MAX_ORDER - i + 1
        cs.append(sum(C[idx + j] * X_MAX ** j for j in range(n_j)))
        idx += n_j
    return cs  # [c0, c1, c2, c3]


def _build_nc(n_chunks, wk):
    """Device kernel: per core, BLK independent [S,S] fp16 output blocks,
    each output tile = sum over n_chunks K=128 bf16 matmuls.

    K=64 matmuls stream at ~1/3 the K=128 rate on TRN2 HW (630ns vs 233ns
    per [128,512]), so the contraction is always presented as K=128:
    a carries [hi; lo] bf16 rows, and when wk == 64 the w rows are
    replicated in SBUF (two DMAs from the same DRAM region) so one matmul
    computes (a_hi + a_lo) @ w."""
    nc = bacc.Bacc(None, target_bir_lowering=False)
    a_d = nc.dram_tensor("a", [BLK, n_chunks, 128, S], mybir.dt.bfloat16,
                         kind="ExternalInput")
    w_d = nc.dram_tensor("w", [BLK, n_chunks, wk, S], mybir.dt.bfloat16,
                         kind="ExternalInput")
    out_d = nc.dram_tensor("out", [BLK, S, S], mybir.dt.float16,
                           kind="ExternalOutput")

    with tile.TileContext(nc) as tc:
        with (
            tc.tile_pool(name="inp", bufs=1) as inp,
            tc.tile_pool(name="ps", bufs=2, space="PSUM") as psp,
            tc.tile_pool(name="outp", bufs=10) as outp,
        ):
            # Prefetch every input tile up front so the steady-state DMA
            # queues carry only output stores.
            a_ts, w_ts = {}, {}
            for blk in range(BLK):
                for c in range(n_chunks):
                    ta = inp.tile([128, S], mybir.dt.bfloat16,
                                  tag=f"a{blk}_{c}")
                    nc.sync.dma_start(ta[:], a_d[blk, c])
                    a_ts[(blk, c)] = ta
                    tw = inp.tile([128, S], mybir.dt.bfloat16,
                                  tag=f"w{blk}_{c}")
                    if wk == 64:
                        nc.sync.dma_start(tw[:64], w_d[blk, c])
                        nc.sync.dma_start(tw[64:], w_d[blk, c])
                    else:
                        nc.sync.dma_start(tw[:], w_d[blk, c])
                    w_ts[(blk, c)] = tw

            # Pool/GpSimd can't read PSUM on TRN2, so whole row-tile drains
            # alternate between DVE (~2.29us) and Act (~1.97us); concurrent
            # same-tile dual-engine drains measured ~10us slower.
            drain_cost = [0.0, 0.0]  # accumulated us on [DVE, Act]
            with nc.allow_low_precision(reason="fp16 out within 2e-2 tol"):
                for blk in range(BLK):
                    for st in range(S // M_TILE):
                        ps = psp.tile([M_TILE, S], mybir.dt.float32, tag="ps")
                        for nt in range(S // N_TILE):
                            for c in range(n_chunks):
                                nc.tensor.matmul(
                                    ps[:, nt * N_TILE:(nt + 1) * N_TILE],
                                    a_ts[(blk, c)][
                                        :, st * M_TILE:(st + 1) * M_TILE],
                                    w_ts[(blk, c)][
                                        :, nt * N_TILE:(nt + 1) * N_TILE],
                                    start=(c == 0),
                                    stop=(c == n_chunks - 1),
                                )
                        ot = outp.tile([M_TILE, S], mybir.dt.float16,
                                       tag="ot")
                        if drain_cost[0] + 2.29 <= drain_cost[1] + 1.97:
                            drain_cost[0] += 2.29
                            nc.vector.tensor_copy(ot[:], ps[:])
                        else:
                            drain_cost[1] += 1.97
                            nc.scalar.copy(ot[:], ps[:])
                        nc.sync.dma_start(
                            out_d[blk, st * M_TILE:(st + 1) * M_TILE, :],
                            ot[:])
    nc.compile()
    return nc


def _get_nc(n_chunks, wk):
    key = (n_chunks, wk)
    if key not in _NC_CACHE:
        _NC_CACHE[key] = _build_nc(n_chunks, wk)
    return _NC_CACHE[key]


def _hilo(v):
    """f32 -> stacked [hi; lo] bf16 rows along axis -2 (hi+lo ~= v)."""
    hi = v.astype(ml_dtypes.bfloat16)
    lo = (v - hi.astype(np.float32)).astype(ml_dtypes.bfloat16)
    return np.concatenate([hi, lo], axis=-2)


def _prepare(x, y, dm, qk):
    """Host prep -> (a, w) bf16 arrays: a [BH, n_chunks, 128, S],
    w [BH, n_chunks, wk, S]."""
    c0, c1, c2, c3 = _coefs()
    yo = (y - OFFSET).astype(np.float32)  # [B,H,D,S]
    if qk:
        n_chunks, wk = 1, D
        at = np.ascontiguousarray(
            (x * dm[None, None, :, :]).transpose(0, 1, 3, 2)
        ).reshape(BH, D, S)
        a = _hilo(at).reshape(BH, 1, 2 * D, S)
        w = (((c3 * yo + c2) * yo + c1) * yo + c0) \
            .astype(ml_dtypes.bfloat16).reshape(BH, 1, D, S)
    else:
        n_chunks, wk = 2, 2 * D
        d = dm[:, 0]
        a = np.empty((BH, 2, 2 * D, S), dtype=ml_dtypes.bfloat16)
        w = np.empty((BH, 2, 2 * D, S), dtype=ml_dtypes.bfloat16)
        xt = x.transpose(0, 1, 3, 2).reshape(BH, D, S)
        di = np.ones_like(d)
        yi = np.ones_like(yo).reshape(BH, D, S)
        yo_r = yo.reshape(BH, D, S)
        for i, ci in enumerate((c0, c1, c2, c3)):
            c, half = divmod(i, 2)
            a[:, c, half * D:(half + 1) * D] = xt * di[None, None, :]
            w[:, c, half * D:(half + 1) * D] = ci * yi
            di = di * d
            yi = yi * yo_r
    return a, w, n_chunks, wk


def kernel(**inputs):
    x = np.asarray(inputs["x"], dtype=np.float32)
    y = np.asarray(inputs["y"], dtype=np.float32)
    dm = np.asarray(inputs["decay_mask"], dtype=np.float32)
    qk = int(np.asarray(inputs["QK_mul"]))

    a, w, n_chunks, wk = _prepare(x, y, dm, qk)
    nc = _get_nc(n_chunks, wk)

    in_maps = [
        {"a": a[c * BLK:(c + 1) * BLK], "w": w[c * BLK:(c + 1) * BLK]}
        for c in range(N_CORES)
    ]
    global _last_nc, _last_in_maps
    _last_nc, _last_in_maps = nc, in_maps

    res = None
    for attempt in range(3):
        try:
            res = run_bass_kernel_spmd(nc, in_maps,
                                       core_ids=list(range(N_CORES)))
            break
        except Exception:
            # transient NRT_EXEC_UNIT_UNRECOVERABLE wedges occur on busy axon
            # terminals; they clear after a pause
            if attempt == 2:
                raise
            import time
            time.sleep(45)

    out = np.empty((BH, S, S), dtype=np.float32)
    for c in range(N_CORES):
        out[c * BLK:(c + 1) * BLK] = res.results[c]["out"]
    return out.reshape(B, H, S, S)


# revision 15
# speedup vs baseline: 1.3253x; 1.1949x over previous
"""Trainium2 Bass kernel for nn_DRAM_MAC_temporal_encoding (polynomial attention).

Math (QK_mul=1):
    out = sum_i coef_i * (x @ (y-OFF)^i) * decay
        = (x * decay) @ P(y-OFF)            # P = Horner cubic, elementwise
so the whole problem is ONE [S,64]@[64,S] matmul per (b,h) head plus the
output write -> memory-bound. The tiny elementwise prep (poly on y,
row-scaling x, transposes, fp16 casts) runs on host; the device does
matmuls + store.

Precision: tolerance is rel_err < 2e-2. fp16 inputs + single fp16 matmul
(fp32 PSUM accumulate) + fp16 output measures 2.5e-4 on the numpy model —
so no hi/lo split and, crucially, the 50 MiB/core fp32 output write
becomes 25 MiB fp16 (host upcasts back to fp32). PSUM->SBUF fp32->fp16
drains rotate across Vector/Scalar/Pool so no single engine bottlenecks.

QK_mul=0: out = sum_i coef_i * ((x*d^i) @ (y-OFF)^i) -> two K=128 chunks
(4 stacked K=64 terms), same kernel with n_chunks=2.

Sharding: 24 (b,h) heads -> 3 per core across 8 cores.
"""

import ml_dtypes
import numpy as np

import concourse.mybir as mybir
import concourse.tile as tile
from concourse import bacc
from concourse.bass_utils import run_bass_kernel_spmd

C = [0.17393044, 0.15653739, 0.14088365, 0.12679529, 5.51975209,
     4.96777688, 4.4709992, -1.44776001, -1.30298401, 46.05483778]
MAX_ORDER = 3
X_MAX = 0.9
OFFSET = 0.45

B, H, S, D = 2, 12, 2048, 64
BH = B * H
N_CORES = 8
BLK = BH // N_CORES  # heads per core

M_TILE = 128   # output rows per matmul (PSUM partitions)
N_TILE = 512   # output cols per matmul (one fp32 PSUM bank)

_NC_CACHE = {}
_last_nc = None
_last_in_maps = None


def _coefs():
    cs = []
    idx = 0
    for i in range(MAX_ORDER + 1):
        n_j = MAX_ORDER - i + 1
        cs.append(sum(C[idx + j] * X_MAX ** j for j in range(n_j)))
        idx += n_j
    return cs  # [c0, c1, c2, c3]


def _build_nc(n_chunks, wk):
    """Device kernel: per core, BLK independent [S,S] fp16 output blocks,
    each output tile = sum over n_chunks K=128 bf16 matmuls.

    K=64 matmuls stream at ~1/3 the K=128 rate on TRN2 HW (630ns vs 233ns
    per [128,512]), so the contraction is always presented as K=128:
    a carries [hi; lo] bf16 rows, and when wk == 64 the w rows are
    replicated in SBUF (two DMAs from the same DRAM region) so one matmul
    computes (a_hi + a_lo) @ w."""
    nc = bacc.Bacc(None, target_bir_lowering=False)
    a_d = nc.dram_tensor("a", [BLK, n_chunks, 128, S], mybir.dt.bfloat16,
                         kind="ExternalInput")
    w_d = nc.dram_tensor("w", [BLK, n_chunks, wk, S], mybir.dt.bfloat16,
                         kind="ExternalInput")
    out_d = nc.dram_tensor("out", [BLK, S, S], mybir.dt.float16,
                           kind="ExternalOutput")

    with tile.TileContext(nc) as tc:
        with (
            tc.tile_pool(name="inp", bufs=1) as inp,
            tc.tile_pool(name="ps", bufs=4, space="PSUM") as psp,
            tc.tile_pool(name="outp", bufs=10) as outp,
        ):
            # Prefetch every input tile up front so the steady-state DMA
            # queues carry only output stores.
            a_ts, w_ts = {}, {}
            for blk in range(BLK):
                for c in range(n_chunks):
                    ta = inp.tile([128, S], mybir.dt.bfloat16,
                                  tag=f"a{blk}_{c}")
                    nc.sync.dma_start(ta[:], a_d[blk, c])
                    a_ts[(blk, c)] = ta
                    tw = inp.tile([128, S], mybir.dt.bfloat16,
                                  tag=f"w{blk}_{c}")
                    if wk == 64:
                        nc.sync.dma_start(tw[:64], w_d[blk, c])
                        nc.sync.dma_start(tw[64:], w_d[blk, c])
                    else:
                        nc.sync.dma_start(tw[:], w_d[blk, c])
                    w_ts[(blk, c)] = tw

            # Pool/GpSimd can't read PSUM on TRN2, so drains go to DVE and
            # Act. A 2-deep ring of [128,2048] PSUM tiles serializes on the
            # ~2.2us whole-tile drain (measured ~2.0us/row-tile cadence);
            # instead use a 4-deep ring of [128,1024] half-tiles whose
            # ~1.1us drains alternate engines.
            HALF = S // 2
            di = 0
            with nc.allow_low_precision(reason="fp16 out within 2e-2 tol"):
                for blk in range(BLK):
                    for st in range(S // M_TILE):
                        ot = outp.tile([M_TILE, S], mybir.dt.float16,
                                       tag="ot")
                        for h in range(2):
                            ps = psp.tile([M_TILE, HALF], mybir.dt.float32,
                                          tag="ps")
                            for ntl in range(HALF // N_TILE):
                                nt = h * (HALF // N_TILE) + ntl
                                for c in range(n_chunks):
                                    nc.tensor.matmul(
                                        ps[:, ntl * N_TILE:
                                           (ntl + 1) * N_TILE],
                                        a_ts[(blk, c)][
                                            :, st * M_TILE:(st + 1) * M_TILE],
                                        w_ts[(blk, c)][
                                            :, nt * N_TILE:(nt + 1) * N_TILE],
                                        start=(c == 0),
                                        stop=(c == n_chunks - 1),
                                    )
                            dst = ot[:, h * HALF:(h + 1) * HALF]
                            if di % 2 == 0:
                                nc.vector.tensor_copy(dst, ps[:])
                            else:
                                nc.scalar.copy(dst, ps[:])
                            di += 1
                        nc.sync.dma_start(
                            out_d[blk, st * M_TILE:(st + 1) * M_TILE, :],
                            ot[:])
    nc.compile()
    return nc


def _get_nc(n_chunks, wk):
    key = (n_chunks, wk)
    if key not in _NC_CACHE:
        _NC_CACHE[key] = _build_nc(n_chunks, wk)
    return _NC_CACHE[key]


def _hilo(v):
    """f32 -> stacked [hi; lo] bf16 rows along axis -2 (hi+lo ~= v)."""
    hi = v.astype(ml_dtypes.bfloat16)
    lo = (v - hi.astype(np.float32)).astype(ml_dtypes.bfloat16)
    return np.concatenate([hi, lo], axis=-2)


def _prepare(x, y, dm, qk):
    """Host prep -> (a, w) bf16 arrays: a [BH, n_chunks, 128, S],
    w [BH, n_chunks, wk, S]."""
    c0, c1, c2, c3 = _coefs()
    yo = (y - OFFSET).astype(np.float32)  # [B,H,D,S]
    if qk:
        n_chunks, wk = 1, D
        at = np.ascontiguousarray(
            (x * dm[None, None, :, :]).transpose(0, 1, 3, 2)
        ).reshape(BH, D, S)
        a = _hilo(at).reshape(BH, 1, 2 * D, S)
        w = (((c3 * yo + c2) * yo + c1) * yo + c0) \
            .astype(ml_dtypes.bfloat16).reshape(BH, 1, D, S)
    else:
        n_chunks, wk = 2, 2 * D
        d = dm[:, 0]
        a = np.empty((BH, 2, 2 * D, S), dtype=ml_dtypes.bfloat16)
        w = np.empty((BH, 2, 2 * D, S), dtype=ml_dtypes.bfloat16)
        xt = x.transpose(0, 1, 3, 2).reshape(BH, D, S)
        di = np.ones_like(d)
        yi = np.ones_like(yo).reshape(BH, D, S)
        yo_r = yo.reshape(BH, D, S)
        for i, ci in enumerate((c0, c1, c2, c3)):
            c, half = divmod(i, 2)
            a[:, c, half * D:(half + 1) * D] = xt * di[None, None, :]
            w[:, c, half * D:(half + 1) * D] = ci * yi
            di = di * d
            yi = yi * yo_r
    return a, w, n_chunks, wk


def kernel(**inputs):
    x = np.asarray(inputs["x"], dtype=np.float32)
    y = np.asarray(inputs["y"], dtype=np.float32)
    dm = np.asarray(inputs["decay_mask"], dtype=np.float32)
    qk = int(np.asarray(inputs["QK_mul"]))

    a, w, n_chunks, wk = _prepare(x, y, dm, qk)
    nc = _get_nc(n_chunks, wk)

    in_maps = [
        {"a": a[c * BLK:(c + 1) * BLK], "w": w[c * BLK:(c + 1) * BLK]}
        for c in range(N_CORES)
    ]
    global _last_nc, _last_in_maps
    _last_nc, _last_in_maps = nc, in_maps

    res = None
    for attempt in range(3):
        try:
            res = run_bass_kernel_spmd(nc, in_maps,
                                       core_ids=list(range(N_CORES)))
            break
        except Exception:
            # transient NRT_EXEC_UNIT_UNRECOVERABLE wedges occur on busy axon
            # terminals; they clear after a pause
            if attempt == 2:
                raise
            import time
            time.sleep(45)

    out = np.empty((BH, S, S), dtype=np.float32)
    for c in range(N_CORES):
        out[c * BLK:(c + 1) * BLK] = res.results[c]["out"]
    return out.reshape(B, H, S, S)


# revision 19
# speedup vs baseline: 1.3664x; 1.0310x over previous
"""Trainium2 Bass kernel for nn_DRAM_MAC_temporal_encoding (polynomial attention).

Math (QK_mul=1):
    out = sum_i coef_i * (x @ (y-OFF)^i) * decay
        = (x * decay) @ P(y-OFF)            # P = Horner cubic, elementwise
so the whole problem is ONE [S,64]@[64,S] matmul per (b,h) head plus the
output write -> memory-bound. The tiny elementwise prep (poly on y,
row-scaling x, transposes, fp16 casts) runs on host; the device does
matmuls + store.

Precision: tolerance is rel_err < 2e-2. fp16 inputs + single fp16 matmul
(fp32 PSUM accumulate) + fp16 output measures 2.5e-4 on the numpy model —
so no hi/lo split and, crucially, the 50 MiB/core fp32 output write
becomes 25 MiB fp16 (host upcasts back to fp32). PSUM->SBUF fp32->fp16
drains rotate across Vector/Scalar/Pool so no single engine bottlenecks.

QK_mul=0: out = sum_i coef_i * ((x*d^i) @ (y-OFF)^i) -> two K=128 chunks
(4 stacked K=64 terms), same kernel with n_chunks=2.

Sharding: 24 (b,h) heads -> 3 per core across 8 cores.
"""

import ml_dtypes
import numpy as np

import concourse.mybir as mybir
import concourse.tile as tile
from concourse import bacc
from concourse.bass_utils import run_bass_kernel_spmd

C = [0.17393044, 0.15653739, 0.14088365, 0.12679529, 5.51975209,
     4.96777688, 4.4709992, -1.44776001, -1.30298401, 46.05483778]
MAX_ORDER = 3
X_MAX = 0.9
OFFSET = 0.45

B, H, S, D = 2, 12, 2048, 64
BH = B * H
N_CORES = 8
BLK = BH // N_CORES  # heads per core

M_TILE = 128   # output rows per matmul (PSUM partitions)
N_TILE = 512   # output cols per matmul (one fp32 PSUM bank)

_NC_CACHE = {}
_last_nc = None
_last_in_maps = None


def _coefs():
    cs = []
    idx = 0
    for i in range(MAX_ORDER + 1):
        n_j = MAX_ORDER - i + 1
        cs.append(sum(C[idx + j] * X_MAX ** j for j in range(n_j)))
        idx += n_j
    return cs  # [c0, c1, c2, c3]


def _build_nc(n_chunks, wk):
    """Device kernel: per core, BLK independent [S,S] fp16 output blocks,
    each output tile = sum over n_chunks K=128 bf16 matmuls.

    K=64 matmuls stream at ~1/3 the K=128 rate on TRN2 HW (630ns vs 233ns
    per [128,512]), so the contraction is always presented as K=128: when
    wk == 64 the upper 64 rows of both operands are memset to zero on the
    Pool engine (no extra HBM traffic) and only 64 real rows upload."""
    nc = bacc.Bacc(None, target_bir_lowering=False)
    a_d = nc.dram_tensor("a", [BLK, n_chunks, wk, S], mybir.dt.bfloat16,
                         kind="ExternalInput")
    w_d = nc.dram_tensor("w", [BLK, n_chunks, wk, S], mybir.dt.bfloat16,
                         kind="ExternalInput")
    out_d = nc.dram_tensor("out", [BLK, S, S], mybir.dt.float16,
                           kind="ExternalOutput")

    with tile.TileContext(nc) as tc:
        with (
            tc.tile_pool(name="inp", bufs=1) as inp,
            tc.tile_pool(name="ps", bufs=4, space="PSUM") as psp,
            tc.tile_pool(name="outp", bufs=10) as outp,
        ):
            # Prefetch every input tile up front so the steady-state DMA
            # queues carry only output stores.
            a_ts, w_ts = {}, {}
            for blk in range(BLK):
                for c in range(n_chunks):
                    ta = inp.tile([128, S], mybir.dt.bfloat16,
                                  tag=f"a{blk}_{c}")
                    nc.sync.dma_start(ta[:wk], a_d[blk, c])
                    a_ts[(blk, c)] = ta
                    tw = inp.tile([128, S], mybir.dt.bfloat16,
                                  tag=f"w{blk}_{c}")
                    nc.sync.dma_start(tw[:wk], w_d[blk, c])
                    w_ts[(blk, c)] = tw
                    if wk < 128:
                        nc.gpsimd.memset(ta[wk:], 0.0)
                        nc.gpsimd.memset(tw[wk:], 0.0)

            # Pool/GpSimd can't read PSUM on TRN2, so drains go to DVE and
            # Act. A 2-deep ring of [128,2048] PSUM tiles serializes on the
            # ~2.2us whole-tile drain (measured ~2.0us/row-tile cadence);
            # instead use a 4-deep ring of [128,1024] half-tiles whose
            # ~1.1us drains alternate engines.
            HALF = S // 2
            di = 0
            with nc.allow_low_precision(reason="fp16 out within 2e-2 tol"):
                for blk in range(BLK):
                    for st in range(S // M_TILE):
                        ot = outp.tile([M_TILE, S], mybir.dt.float16,
                                       tag="ot")
                        for h in range(2):
                            ps = psp.tile([M_TILE, HALF], mybir.dt.float32,
                                          tag="ps")
                            for ntl in range(HALF // N_TILE):
                                nt = h * (HALF // N_TILE) + ntl
                                for c in range(n_chunks):
                                    nc.tensor.matmul(
                                        ps[:, ntl * N_TILE:
                                           (ntl + 1) * N_TILE],
                                        a_ts[(blk, c)][
                                            :, st * M_TILE:(st + 1) * M_TILE],
                                        w_ts[(blk, c)][
                                            :, nt * N_TILE:(nt + 1) * N_TILE],
                                        start=(c == 0),
                                        stop=(c == n_chunks - 1),
                                    )
                            dst = ot[:, h * HALF:(h + 1) * HALF]
                            if di % 2 == 0:
                                nc.vector.tensor_copy(dst, ps[:])
                            else:
                                nc.scalar.copy(dst, ps[:])
                            di += 1
                        nc.sync.dma_start(
                            out_d[blk, st * M_TILE:(st + 1) * M_TILE, :],
                            ot[:])
    nc.compile()
    return nc


def _get_nc(n_chunks, wk):
    key = (n_chunks, wk)
    if key not in _NC_CACHE:
        _NC_CACHE[key] = _build_nc(n_chunks, wk)
    return _NC_CACHE[key]


def _prepare(x, y, dm, qk):
    """Host prep -> (a, w) bf16 arrays: a [BH, n_chunks, 128, S],
    w [BH, n_chunks, wk, S]."""
    c0, c1, c2, c3 = _coefs()
    yo = (y - OFFSET).astype(np.float32)  # [B,H,D,S]
    if qk:
        n_chunks, wk = 1, D
        a = (x * dm[None, None, :, :]).transpose(0, 1, 3, 2) \
            .astype(ml_dtypes.bfloat16).reshape(BH, 1, D, S)
        w = (((c3 * yo + c2) * yo + c1) * yo + c0) \
            .astype(ml_dtypes.bfloat16).reshape(BH, 1, D, S)
    else:
        n_chunks, wk = 2, 2 * D
        d = dm[:, 0]
        a = np.empty((BH, 2, 2 * D, S), dtype=ml_dtypes.bfloat16)
        w = np.empty((BH, 2, 2 * D, S), dtype=ml_dtypes.bfloat16)
        xt = x.transpose(0, 1, 3, 2).reshape(BH, D, S)
        di = np.ones_like(d)
        yi = np.ones_like(yo).reshape(BH, D, S)
        yo_r = yo.reshape(BH, D, S)
        for i, ci in enumerate((c0, c1, c2, c3)):
            c, half = divmod(i, 2)
            a[:, c, half * D:(half + 1) * D] = xt * di[None, None, :]
            w[:, c, half * D:(half + 1) * D] = ci * yi
            di = di * d
            yi = yi * yo_r
    return a, w, n_chunks, wk


def kernel(**inputs):
    x = np.asarray(inputs["x"], dtype=np.float32)
    y = np.asarray(inputs["y"], dtype=np.float32)
    dm = np.asarray(inputs["decay_mask"], dtype=np.float32)
    qk = int(np.asarray(inputs["QK_mul"]))

    a, w, n_chunks, wk = _prepare(x, y, dm, qk)
    nc = _get_nc(n_chunks, wk)

    in_maps = [
        {"a": a[c * BLK:(c + 1) * BLK], "w": w[c * BLK:(c + 1) * BLK]}
        for c in range(N_CORES)
    ]
    global _last_nc, _last_in_maps
    _last_nc, _last_in_maps = nc, in_maps

    res = None
    for attempt in range(3):
        try:
            res = run_bass_kernel_spmd(nc, in_maps,
                                       core_ids=list(range(N_CORES)))
            break
        except Exception:
            # transient NRT_EXEC_UNIT_UNRECOVERABLE wedges occur on busy axon
            # terminals; they clear after a pause
            if attempt == 2:
                raise
            import time
            time.sleep(45)

    out = np.empty((BH, S, S), dtype=np.float32)
    for c in range(N_CORES):
        out[c * BLK:(c + 1) * BLK] = res.results[c]["out"]
    return out.reshape(B, H, S, S)


# revision 21
# speedup vs baseline: 1.3757x; 1.0068x over previous
"""Trainium2 Bass kernel for nn_DRAM_MAC_temporal_encoding (polynomial attention).

Math (QK_mul=1):
    out = sum_i coef_i * (x @ (y-OFF)^i) * decay
        = (x * decay) @ P(y-OFF)            # P = Horner cubic, elementwise
so the whole problem is ONE [S,64]@[64,S] matmul per (b,h) head plus the
output write -> memory-bound. The tiny elementwise prep (poly on y,
row-scaling x, transposes, fp16 casts) runs on host; the device does
matmuls + store.

Precision: tolerance is rel_err < 2e-2. fp16 inputs + single fp16 matmul
(fp32 PSUM accumulate) + fp16 output measures 2.5e-4 on the numpy model —
so no hi/lo split and, crucially, the 50 MiB/core fp32 output write
becomes 25 MiB fp16 (host upcasts back to fp32). PSUM->SBUF fp32->fp16
drains rotate across Vector/Scalar/Pool so no single engine bottlenecks.

QK_mul=0: out = sum_i coef_i * ((x*d^i) @ (y-OFF)^i) -> two K=128 chunks
(4 stacked K=64 terms), same kernel with n_chunks=2.

Sharding: 24 (b,h) heads -> 3 per core across 8 cores.
"""

import ml_dtypes
import numpy as np

import concourse.mybir as mybir
import concourse.tile as tile
from concourse import bacc
from concourse.bass_utils import run_bass_kernel_spmd

C = [0.17393044, 0.15653739, 0.14088365, 0.12679529, 5.51975209,
     4.96777688, 4.4709992, -1.44776001, -1.30298401, 46.05483778]
MAX_ORDER = 3
X_MAX = 0.9
OFFSET = 0.45

B, H, S, D = 2, 12, 2048, 64
BH = B * H
N_CORES = 8
BLK = BH // N_CORES  # heads per core

M_TILE = 128   # output rows per matmul (PSUM partitions)
N_TILE = 512   # output cols per matmul (one fp32 PSUM bank)

_NC_CACHE = {}
_last_nc = None
_last_in_maps = None


def _coefs():
    cs = []
    idx = 0
    for i in range(MAX_ORDER + 1):
        n_j = MAX_ORDER - i + 1
        cs.append(sum(C[idx + j] * X_MAX ** j for j in range(n_j)))
        idx += n_j
    return cs  # [c0, c1, c2, c3]


def _build_nc(n_chunks, wk):
    """Device kernel: per core, BLK independent [S,S] fp16 output blocks,
    each output tile = sum over n_chunks K=128 bf16 matmuls.

    K=64 matmuls stream at ~1/3 the K=128 rate on TRN2 HW (630ns vs 233ns
    per [128,512]), so the contraction is always presented as K=128: when
    wk == 64 the upper 64 rows of both operands are memset to zero on the
    Pool engine (no extra HBM traffic) and only 64 real rows upload."""
    nc = bacc.Bacc(None, target_bir_lowering=False)
    a_d = nc.dram_tensor("a", [BLK, n_chunks, wk, S], mybir.dt.bfloat16,
                         kind="ExternalInput")
    w_d = nc.dram_tensor("w", [BLK, n_chunks, wk, S], mybir.dt.bfloat16,
                         kind="ExternalInput")
    out_d = nc.dram_tensor("out", [BLK, S, S], mybir.dt.float16,
                           kind="ExternalOutput")

    with tile.TileContext(nc) as tc:
        with (
            tc.tile_pool(name="inp", bufs=1) as inp,
            tc.tile_pool(name="ps", bufs=4, space="PSUM") as psp,
            tc.tile_pool(name="outp", bufs=10) as outp,
        ):
            # Input tiles: zero rows wk:128 are memset up front (blk0's w on
            # DVE, which is idle until drains start; the rest on Pool), and
            # each head's loads are emitted just before its row-tiles so the
            # single DMA FIFO starts storing after only one head's loads.
            a_ts, w_ts = {}, {}
            for blk in range(BLK):
                for c in range(n_chunks):
                    ta = inp.tile([128, S], mybir.dt.bfloat16,
                                  name=f"a{blk}_{c}", tag=f"a{blk}_{c}")
                    a_ts[(blk, c)] = ta
                    tw = inp.tile([128, S], mybir.dt.bfloat16,
                                  name=f"w{blk}_{c}", tag=f"w{blk}_{c}")
                    w_ts[(blk, c)] = tw
            if wk < 128:
                for blk in range(BLK):
                    for c in range(n_chunks):
                        nc.gpsimd.memset(a_ts[(blk, c)][wk:], 0.0)
                        eng = nc.vector if blk == 0 else nc.gpsimd
                        eng.memset(w_ts[(blk, c)][wk:], 0.0)

            # Pool/GpSimd can't read PSUM on TRN2, so drains go to DVE and
            # Act. A 2-deep ring of [128,2048] PSUM tiles serializes on the
            # ~2.2us whole-tile drain (measured ~2.0us/row-tile cadence);
            # instead use a 4-deep ring of [128,1024] half-tiles whose
            # ~1.1us drains alternate engines.
            HALF = S // 2
            di = 0
            with nc.allow_low_precision(reason="fp16 out within 2e-2 tol"):
                for blk in range(BLK):
                    for c in range(n_chunks):
                        nc.sync.dma_start(a_ts[(blk, c)][:wk], a_d[blk, c])
                        nc.sync.dma_start(w_ts[(blk, c)][:wk], w_d[blk, c])
                    for st in range(S // M_TILE):
                        ot = outp.tile([M_TILE, S], mybir.dt.float16,
                                       tag="ot")
                        for h in range(2):
                            ps = psp.tile([M_TILE, HALF], mybir.dt.float32,
                                          tag="ps")
                            for ntl in range(HALF // N_TILE):
                                nt = h * (HALF // N_TILE) + ntl
                                for c in range(n_chunks):
                                    nc.tensor.matmul(
                                        ps[:, ntl * N_TILE:
                                           (ntl + 1) * N_TILE],
                                        a_ts[(blk, c)][
                                            :, st * M_TILE:(st + 1) * M_TILE],
                                        w_ts[(blk, c)][
                                            :, nt * N_TILE:(nt + 1) * N_TILE],
                                        start=(c == 0),
                                        stop=(c == n_chunks - 1),
                                    )
                            dst = ot[:, h * HALF:(h + 1) * HALF]
                            if di % 2 == 0:
                                nc.vector.tensor_copy(dst, ps[:])
                            else:
                                nc.scalar.copy(dst, ps[:])
                            di += 1
                        nc.sync.dma_start(
                            out_d[blk, st * M_TILE:(st + 1) * M_TILE, :],
                            ot[:])
    nc.compile()
    return nc


def _get_nc(n_chunks, wk):
    key = (n_chunks, wk)
    if key not in _NC_CACHE:
        _NC_CACHE[key] = _build_nc(n_chunks, wk)
    return _NC_CACHE[key]


def _prepare(x, y, dm, qk):
    """Host prep -> (a, w) bf16 arrays: a [BH, n_chunks, 128, S],
    w [BH, n_chunks, wk, S]."""
    c0, c1, c2, c3 = _coefs()
    yo = (y - OFFSET).astype(np.float32)  # [B,H,D,S]
    if qk:
        n_chunks, wk = 1, D
        a = (x * dm[None, None, :, :]).transpose(0, 1, 3, 2) \
            .astype(ml_dtypes.bfloat16).reshape(BH, 1, D, S)
        w = (((c3 * yo + c2) * yo + c1) * yo + c0) \
            .astype(ml_dtypes.bfloat16).reshape(BH, 1, D, S)
    else:
        n_chunks, wk = 2, 2 * D
        d = dm[:, 0]
        a = np.empty((BH, 2, 2 * D, S), dtype=ml_dtypes.bfloat16)
        w = np.empty((BH, 2, 2 * D, S), dtype=ml_dtypes.bfloat16)
        xt = x.transpose(0, 1, 3, 2).reshape(BH, D, S)
        di = np.ones_like(d)
        yi = np.ones_like(yo).reshape(BH, D, S)
        yo_r = yo.reshape(BH, D, S)
        for i, ci in enumerate((c0, c1, c2, c3)):
            c, half = divmod(i, 2)
            a[:, c, half * D:(half + 1) * D] = xt * di[None, None, :]
            w[:, c, half * D:(half + 1) * D] = ci * yi
            di = di * d
            yi = yi * yo_r
    return a, w, n_chunks, wk


def kernel(**inputs):
    x = np.asarray(inputs["x"], dtype=np.float32)
    y = np.asarray(inputs["y"], dtype=np.float32)
    dm = np.asarray(inputs["decay_mask"], dtype=np.float32)
    qk = int(np.asarray(inputs["QK_mul"]))

    a, w, n_chunks, wk = _prepare(x, y, dm, qk)
    nc = _get_nc(n_chunks, wk)

    in_maps = [
        {"a": a[c * BLK:(c + 1) * BLK], "w": w[c * BLK:(c + 1) * BLK]}
        for c in range(N_CORES)
    ]
    global _last_nc, _last_in_maps
    _last_nc, _last_in_maps = nc, in_maps

    res = None
    for attempt in range(3):
        try:
            res = run_bass_kernel_spmd(nc, in_maps,
                                       core_ids=list(range(N_CORES)))
            break
        except Exception:
            # transient NRT_EXEC_UNIT_UNRECOVERABLE wedges occur on busy axon
            # terminals; they clear after a pause
            if attempt == 2:
                raise
            import time
            time.sleep(45)

    out = np.empty((BH, S, S), dtype=np.float32)
    for c in range(N_CORES):
        out[c * BLK:(c + 1) * BLK] = res.results[c]["out"]
    return out.reshape(B, H, S, S)
